# revision 1
# baseline (speedup 1.0000x reference)
"""Trainium2 Bass kernel for the BiDAF-style attention-flow layer.

S[b,t,j] = H.w_h + U.w_u + (H*w_hu).U + bias
c2q      = softmax_j(S) @ U
q2c      = softmax_t(max_j S) @ H   (broadcast over t)
out      = concat([H, c2q, H*c2q, H*q2c], axis=-1)

Sharding: data-parallel over batch B=64 across 8 NeuronCores (8 batches per
core); W/b replicated; no collectives.

Math notes (exact up to fp rounding):
 - softmax_j(S) is invariant to the per-row sH[t] and bias terms, so the
   c2q path uses exp(S_core + sU + b) only.
 - q2c logits m[t] = max_j S[t,:] = sH[t] + max_j(S_core + sU + b); the
   softmax over t is computed without max-subtraction (logits are O(5),
   safe in fp32).
"""

import numpy as np

import concourse.bass as bass
import concourse.mybir as mybir
import concourse.tile as tile
from concourse.bass_utils import run_bass_kernel_spmd
from concourse.masks import make_identity

B, T, J, D = 64, 1024, 64, 256
NCORES = 8
BL = B // NCORES  # batches per core
NT = T // 128     # t-tiles per batch
F32 = mybir.dt.float32
BF16 = mybir.dt.bfloat16
F32R = mybir.dt.float32r
AX = mybir.AxisListType.X
AF = mybir.ActivationFunctionType


def split_multi_waits(nc, max_waits=1):
    """Walrus in this container rejects instructions with more than a couple
    of embedded sync waits. Hoist extras into standalone EventSemaphore
    instructions right before the offending instruction."""
    n = 0
    for fn in nc.m.functions:
        for bb in fn.blocks:
            new_insts = []
            for inst in bb.instructions:
                si = getattr(inst, "sync_info", None)
                if si is not None and si.on_wait and len(si.on_wait) > max_waits:
                    waits = list(si.on_wait)
                    for w in waits[:-max_waits]:
                        n += 1
                        ev = mybir.InstEventSemaphore(
                            name=f"I-wsplit-{n}", ins=[], outs=[]
                        )
                        ev.engine = inst.engine
                        ev.sync_info = mybir.SyncInfo(on_wait=[w], on_update=[])
                        new_insts.append(ev)
                    inst.sync_info = mybir.SyncInfo(
                        on_wait=waits[-max_waits:], on_update=list(si.on_update)
                    )
                new_insts.append(inst)
            bb.instructions[:] = new_insts
    return n


def build_nc():
    nc = bass.Bass()
    H = nc.declare_dram_parameter("H", [BL, T, D], F32, isOutput=False)
    U = nc.declare_dram_parameter("U", [BL, J, D], F32, isOutput=False)
    W = nc.declare_dram_parameter("W", [3 * D], F32, isOutput=False)
    b = nc.declare_dram_parameter("b", [1], F32, isOutput=False)
    out = nc.declare_dram_parameter("out", [BL, T, 4 * D], F32, isOutput=True)

    with tile.TileContext(nc) as tc:
        with (
            tc.tile_pool(name="singles", bufs=1) as singles,
            tc.tile_pool(name="batch", bufs=2) as bpool,
            tc.tile_pool(name="outp", bufs=3) as outp,
            tc.tile_pool(name="small", bufs=4) as small,
            tc.tile_pool(name="ps_ht", bufs=2, space="PSUM") as ps_ht,
            tc.tile_pool(name="ps_s", bufs=2, space="PSUM") as ps_s,
            tc.tile_pool(name="ps_et", bufs=1, space="PSUM") as ps_et,
            tc.tile_pool(name="ps_c2q", bufs=2, space="PSUM") as ps_c2q,
            tc.tile_pool(name="ps_q2c", bufs=1, space="PSUM") as ps_q2c,
        ):
            ident = singles.tile([128, 128], F32)
            make_identity(nc, ident[:])
            ident_bf = singles.tile([128, 128], BF16)
            make_identity(nc, ident_bf[:])
            ones_row_bf = singles.tile([1, 128], BF16)
            nc.vector.memset(ones_row_bf[:], 1.0)

            # w_u broadcast over 64 partitions (for the sU reduction)
            w_u_bc = singles.tile([J, D], F32)
            wsl = W[D : 2 * D]
            nc.sync.dma_start(
                out=w_u_bc[:],
                in_=bass.AP(tensor=wsl.tensor, offset=wsl.offset,
                            ap=[[0, J]] + list(wsl.ap)),
            )
            # w_hu and w_h as [128,1] column blocks
            whu_col = singles.tile([128, 2], F32)
            wh_col = singles.tile([128, 2], F32)
            for k in range(2):
                nc.sync.dma_start(
                    out=whu_col[:, k : k + 1],
                    in_=W[2 * D + 128 * k : 2 * D + 128 * (k + 1)].rearrange(
                        "(p o) -> p o", o=1
                    ),
                )
                nc.sync.dma_start(
                    out=wh_col[:, k : k + 1],
                    in_=W[128 * k : 128 * (k + 1)].rearrange("(p o) -> p o", o=1),
                )
            b_sb = singles.tile([1, 1], F32)
            nc.sync.dma_start(out=b_sb[:], in_=b[:].rearrange("(p o) -> p o", o=1))

            SEG = 4 * D + 1

            def alloc_and_load(bi):
                ot_b = outp.tile([128, NT * SEG], F32, tag="ot")
                ot3 = ot_b[:].rearrange("p (n c) -> p n c", n=NT)
                nc.gpsimd.memset(ot3[:, :, 0:1], 1.0)
                nc.sync.dma_start(
                    out=ot3[:, :, 1 : D + 1],
                    in_=H[bi].rearrange("(n p) d -> p n d", p=128),
                )
                return ot3

            ot3_next = alloc_and_load(0)

            for bi in range(BL):
                # ---- per-batch setup -------------------------------------
                ot3 = ot3_next
                if bi + 1 < BL:
                    ot3_next = alloc_and_load(bi + 1)
                U_ext = bpool.tile([J, D + 1], F32, tag="uext")
                nc.sync.dma_start(out=U_ext[:, 0:D], in_=U[bi])
                nc.vector.memset(U_ext[:, D : D + 1], 1.0)

                su_scr = bpool.tile([J, D], F32, tag="suscr")
                su_col = bpool.tile([J, 1], F32, tag="sucol")
                nc.vector.tensor_mul(su_scr[:], U_ext[:, 0:D], w_u_bc[:])
                nc.vector.reduce_sum(su_col[:], su_scr[:], axis=AX,
                                     op=mybir.AluOpType.add)

                # transposes of U blocks and sU into one PSUM tile
                utp = ps_ht.tile([128, 192], F32, tag="ht")
                nc.tensor.transpose(utp[:, 0:64], U_ext[:, 0:128], ident[0:J, 0:J])
                nc.tensor.transpose(utp[:, 64:128], U_ext[:, 128:256], ident[0:J, 0:J])
                nc.tensor.transpose(utp[0:1, 128:192], su_col[:], ident[0:J, 0:J])

                # rhs_s[k] = [w_hu * U^T | w_h]  (two 65-col blocks, bf16)
                rhs_s = bpool.tile([128, 130], BF16, tag="rhs")
                for k in range(2):
                    nc.vector.tensor_scalar_mul(
                        rhs_s[:, 65 * k : 65 * k + 64],
                        utp[:, 64 * k : 64 * k + 64],
                        whu_col[:, k : k + 1],
                    )
                    nc.scalar.copy(
                        rhs_s[:, 65 * k + 64 : 65 * k + 65], wh_col[:, k : k + 1]
                    )
                su_ext = bpool.tile([1, 65], BF16, tag="suext")
                nc.vector.tensor_scalar_add(
                    su_ext[0:1, 0:64], utp[0:1, 128:192], b_sb[0:1, 0:1]
                )
                nc.vector.memset(su_ext[0:1, 64:65], 0.0)
                u_bf = bpool.tile([J, D + 1], BF16, tag="ubf")
                nc.vector.tensor_copy(u_bf[:], U_ext[:])

                # ---- pass 1 over t-tiles ---------------------------------
                # ot3: whole batch of NT output tiles, each [128, 1025] laid
                # out as [ones | H | c2q | H*c2q | H*q2c]; H prefetched one
                # batch ahead, stores per segment on the scalar HWDGE queue.
                q2czt = ps_q2c.tile([1, 257], F32, tag="q2czt")
                for ti in range(NT):
                    ot = ot3[:, ti]
                    hb = small.tile([128, D + 1], BF16, tag="hb")
                    nc.vector.tensor_copy(hb[:], ot[:, 0 : D + 1])
                    htp = ps_ht.tile([128, 256], BF16, tag="ht")
                    nc.tensor.transpose(htp[:, 0:128], hb[:, 1:129], ident_bf[:])
                    nc.tensor.transpose(htp[:, 128:256], hb[:, 129:257], ident_bf[:])
                    ht_sb = small.tile([128, 256], BF16, tag="htsb")
                    nc.scalar.copy(ht_sb[:], htp[:])

                    sp = ps_s.tile([128, 65], F32, tag="s")
                    nc.tensor.matmul(
                        sp[:], ht_sb[:, 0:128], rhs_s[:, 0:65],
                        start=True, stop=False,
                    )
                    nc.tensor.matmul(
                        sp[:], ht_sb[:, 128:256], rhs_s[:, 65:130],
                        start=False, stop=False,
                    )
                    nc.tensor.matmul(
                        sp[:], ones_row_bf[:], su_ext[:], start=False, stop=True
                    )

                    r = small.tile([128, 1], F32, tag="r")
                    nc.vector.reduce_max(r[:], sp[:, 0:64], axis=AX,
                                         op=mybir.AluOpType.max)
                    E = small.tile([128, 64], BF16, tag="E")
                    nc.scalar.activation(E[:], sp[:, 0:64], AF.Exp)
                    em = small.tile([128, 1], BF16, tag="em")
                    nc.scalar.activation(em[:], sp[:, 64:65], AF.Exp,
                                         bias=r[:], scale=1.0)

                    etp = ps_et.tile([J, 128], BF16, tag="et")
                    nc.tensor.transpose(etp[:], E[:], ident_bf[:])
                    et_sb = small.tile([J, 128], BF16, tag="etsb")
                    nc.scalar.copy(et_sb[:], etp[:])

                    cq = ps_c2q.tile([128, 257], F32, tag="c2q")
                    nc.tensor.matmul(
                        cq[:], et_sb[:], u_bf[:],
                        start=True, stop=True,
                    )
                    zinv = small.tile([128, 1], F32, tag="zinv")
                    nc.vector.reciprocal(zinv[:], cq[:, 256:257])
                    nc.scalar.activation(ot[:, D + 1 : 2 * D + 1], cq[:, 0:D],
                                         AF.Copy, scale=zinv[:])
                    nc.vector.scalar_tensor_tensor(
                        out=ot[:, 2 * D + 1 : 3 * D + 1], in0=cq[:, 0:D],
                        scalar=zinv[:], in1=ot[:, 1 : D + 1],
                        op0=mybir.AluOpType.mult, op1=mybir.AluOpType.mult,
                    )

                    # q2czt col0 = sum(em) (via the ones column), cols 1:257 =
                    # sum(em * H) -- one accumulation group, one PSUM bank.
                    nc.tensor.matmul(
                        q2czt[0:1, 0:257], em[:], hb[:],
                        start=(ti == 0), stop=(ti == NT - 1),
                        skip_group_check=True,
                    )

                # ---- q2c broadcast + pass 2 ------------------------------
                ztinv = bpool.tile([1, 1], F32, tag="ztinv")
                nc.vector.reciprocal(ztinv[:], q2czt[0:1, 0:1])
                q2c_row = bpool.tile([1, 256], BF16, tag="q2crow")
                nc.vector.tensor_scalar_mul(q2c_row[:], q2czt[0:1, 1:257], ztinv[:])
                q2cbp = ps_et.tile([128, 256], F32, tag="et")
                nc.tensor.matmul(q2cbp[:], ones_row_bf[:], q2c_row[:],
                                 start=True, stop=True)
                q2cb = bpool.tile([128, 256], F32, tag="q2cb")
                nc.scalar.copy(q2cb[:], q2cbp[:])

                for ti in range(NT):
                    ot = ot3[:, ti]
                    nc.gpsimd.tensor_mul(
                        ot[:, 3 * D + 1 : 4 * D + 1], ot[:, 1 : D + 1], q2cb[:]
                    )
                    nc.scalar.dma_start(
                        out=out[bi, 128 * ti : 128 * (ti + 1), :],
                        in_=ot[:, 1 : 4 * D + 1],
                    )

    split_multi_waits(nc)
    return nc


_NC_CACHE = None


def get_nc():
    global _NC_CACHE
    if _NC_CACHE is None:
        _NC_CACHE = build_nc()
    return _NC_CACHE


def make_in_maps(H, U, W, b):
    H = np.ascontiguousarray(np.asarray(H, dtype=np.float32))
    U = np.ascontiguousarray(np.asarray(U, dtype=np.float32))
    W = np.ascontiguousarray(np.asarray(W, dtype=np.float32))
    b = np.ascontiguousarray(np.asarray(b, dtype=np.float32))
    return [
        {
            "H": H[i * BL : (i + 1) * BL],
            "U": U[i * BL : (i + 1) * BL],
            "W": W,
            "b": b,
        }
        for i in range(NCORES)
    ]


def kernel(H, U, W, b):
    nc = get_nc()
    in_maps = make_in_maps(H, U, W, b)
    res = run_bass_kernel_spmd(nc, in_maps, core_ids=list(range(NCORES)))
    return np.concatenate([res.results[i]["out"] for i in range(NCORES)], axis=0)

